# revision 19
# baseline (speedup 1.0000x reference)
"""GQA sliding-window attention (training path, no causal mask, no 1/sqrt(d)
scaling) on 8 Trainium2 NeuronCores.

Reference semantics (see original nn.Module):
  q = x@Wq+bq [b,s,16,64]; k,v = x@Wk+bk / x@Wv+bv [b,s,2,64]
  k,v zero-padded by 128 on both sides of s; query i attends padded
  positions [i, i+256); padded positions contribute score 0 (exp->1)
  and value 0. out = attn @ Wo + bo.

Sharding: batch x sequence. 8 shards = 2 batches x 4 chunks of 512 query
rows. Each core receives x^T for its 512 rows plus a 128-row halo on each
side (zero rows outside [0, 2048)), with an appended 0/1 validity row so
that K/V bias is only added at in-range positions. Host gathers per-core
outputs; no collectives.

Datapath: fp16 for x/Wq/Wk/Wv/qT/kT (10-bit mantissa keeps score precision
at tf32 level), bf16 for probs/V (exp outputs exceed fp16 range), f32r for
attnT/Wo/normalization. All matmuls stream 1 cycle/column; PSUM stays f32.

Per-core dataflow:
  K/V projections accumulate per 128-row contraction chunk as x^T chunks
  stream in, so the PE starts ~1.5us after launch. V transpose + Q
  projection follow (wq streams during K/V). Attention processes HEAD
  PAIRS sharing a kv group: scores for tile t, chunk c=j-t are
  S^T[128kv, (128q x 2 heads)] = kT_j^T @ qT_pair (N=256), one exp per
  (pair, tile) into bf16 pt, band masking via one precomputed triangle
  mask on the DVE per half, then PV matmuls (N=256) accumulate a
  head-interleaved [65, 512x2] PSUM pv (row 64 = softmax denominator via a
  ones-column in V); tiles t<2 live in bank A, t>=2 in bank B so each bank
  is one accumulation group (start=True clears has_written for the whole
  2KB zero region - one start per bank). Normalization (DVE
  reciprocal_approx_fast + two K=1 PE broadcasts via sel3 + DVE multiply)
  is interleaved in waves; denominator rows for both kv groups collect via
  direct DVE copies at legal partition bases (den[128, G, 3, SQ] - no DMA
  bounces). The Wo projection streams out per 128-column block.
"""

import numpy as np

DIM = 1024
NH = 16  # query heads
G = 2  # kv heads
HD = 64  # head dim
W = 256  # window
HALF = 128
BATCH, SEQ = 2, 2048
NCORES = 8
SQ = 512  # query rows per core
SK = SQ + 2 * HALF  # 768 kv halo rows per core
KC = DIM // 128  # 8 contraction chunks
NJ = SK // 128  # 6 kv chunks
NT = SQ // 128  # 4 q tiles

_CACHE = {}


def _build_program(dbg=False):
    import concourse.bass as bass
    import concourse.mybir as mybir
    import concourse.tile as tile
    from concourse import bacc

    f32 = mybir.dt.float32
    f32r = mybir.dt.float32r
    f16 = mybir.dt.float16
    bf16 = mybir.dt.bfloat16

    nc = bacc.Bacc("TRN2", target_bir_lowering=False, debug=False, num_devices=NCORES)
    dbg_t = {}
    if dbg:
        for name, shape, dt in [
            ("dbg_qT", [128, KC, SQ], f16), ("dbg_kT", [128, SK], f16),
            ("dbg_vt", [128, NJ, G, HD + 1], bf16),
            ("dbg_pt", [128, 8, NT, 3, 2, 128], bf16),
            ("dbg_attnT", [128, KC, SQ], bf16),
            ("dbg_den", [128, G, 3, SQ], f32),
        ]:
            dbg_t[name] = nc.declare_dram_parameter(name, shape, dt, isOutput=True)

    # All weight params are pre-packed on the host so every DMA lands as
    # contiguous-per-partition segments >= 1.5KB (256B-segment rearrange
    # DMAs ran the input queues at ~28GB/s vs ~178GB/s).
    xaT = nc.declare_dram_parameter("xaT", [DIM + 1, SK], f16, isOutput=False)
    wq = nc.declare_dram_parameter("wq", [KC, 128, KC, 128], f16, isOutput=False)
    wk = nc.declare_dram_parameter("wk", [128, KC, G * HD], f16, isOutput=False)
    wv = nc.declare_dram_parameter("wv", [128, KC, G * HD], f16, isOutput=False)
    wka = nc.declare_dram_parameter("wka", [1, G * HD], f16, isOutput=False)
    wva = nc.declare_dram_parameter("wva", [1, G * HD], f16, isOutput=False)
    wo = nc.declare_dram_parameter("wo", [DIM, DIM], bf16, isOutput=False)
    bqbo = nc.declare_dram_parameter("bqbo", [128, 2 * KC], f32, isOutput=False)
    # packed bf16 constants: [ones(2) | ident(128) | mask(2*2*2*128) | sel3(2*128)]
    cbf = nc.declare_dram_parameter("cbf", [128, 2 + 128 + 1024 + 256], bf16, isOutput=False)
    yT = nc.declare_dram_parameter("yT", [DIM, SQ], bf16, isOutput=True)

    with tile.TileContext(nc) as tc:
        with (
            nc.allow_low_precision("fp16/bf16 matmul inputs; accumulation stays fp32"),
            tc.tile_pool(name="wts", bufs=1) as wts,
            tc.tile_pool(name="sb", bufs=1) as sb,
            tc.tile_pool(name="pt", bufs=3) as ptp,
            tc.tile_pool(name="yst", bufs=2) as yst,
            tc.tile_pool(name="pA", bufs=2, space="PSUM") as poolA,
            tc.tile_pool(name="pv", bufs=2, space="PSUM") as pvP,
        ):
            # ---- constant loads ----
            # sync: wk, xaug, xT evens, wq evens (+ den bounces, y stores later)
            # scalar: wv, augs, xT odds, small constants, wq odds (+ y stores)
            # gpsimd: packed bf16 consts, wo (needed late)
            wk_sb = wts.tile([128, KC, G * HD], f16, tag="wk")
            wv_sb = wts.tile([128, KC, G * HD], f16, tag="wv")
            xaug = wts.tile([1, SK], f16, tag="xaug")
            wk_aug = wts.tile([1, G * HD], f16, tag="wkaug")
            wv_aug = wts.tile([1, G * HD], f16, tag="wvaug")
            xT_sb = wts.tile([128, KC, SK], f16, tag="xT")
            wq_sb = wts.tile([128, KC, KC, 128], f16, tag="wq")  # [p, dd, kc, c]
            wo_sb = wts.tile([128, KC, DIM], bf16, tag="wo")
            bqbo_sb = wts.tile([128, 2 * KC], f32, tag="bqbo")
            bq_sb = bqbo_sb[:, 0:KC]
            bo_sb = bqbo_sb[:, KC:2 * KC]
            cbf_sb = wts.tile([128, 2 + 128 + 1024 + 256], bf16, tag="cbf")
            ones_sb = cbf_sb[:, 0:2]
            ident = cbf_sb[:, 2:130]
            mask_sb = cbf_sb[:, 130:1154].rearrange(
                "p (a b c d) -> p a b c d", a=2, b=2, c=2)
            sel3_sb = cbf_sb[:, 1154:1410].rearrange("p (a b) -> p a b", a=2)

            # Few large DMAs per queue: each dma_start costs ~600-900ns of
            # issue time on its engine and rings only keep ~4 in flight, so
            # the 33-DMA schedule was issue-paced, not bandwidth-paced.
            # sync:   wk, xT chunks 0:4, wq halves  (+ y stores later)
            # scalar: wv, xT chunks 4:8, bqbo, sel3, wq halves (+ y stores)
            # gpsimd: cbf, aug rows, wo
            nc.sync.dma_start(out=xT_sb[:, 0, :], in_=xaT[0:128, :])
            nc.sync.dma_start(out=wk_sb[:, :, :], in_=wk[:, :, :])
            nc.scalar.dma_start(out=wv_sb[:, :, :], in_=wv[:, :, :])
            nc.sync.dma_start(
                out=xT_sb[:, 1:4, :],
                in_=xaT[128:512, :].rearrange("(a p) w -> p a w", p=128))
            nc.scalar.dma_start(
                out=xT_sb[:, 4:8, :],
                in_=xaT[512:1024, :].rearrange("(a p) w -> p a w", p=128))
            nc.gpsimd.dma_start(out=cbf_sb[:, :], in_=cbf[:, :])
            nc.gpsimd.dma_start(out=xaug[:, :], in_=xaT[DIM:DIM + 1, :])
            nc.gpsimd.dma_start(out=wk_aug[:, :], in_=wka[:, :])
            nc.gpsimd.dma_start(out=wv_aug[:, :], in_=wva[:, :])
            nc.scalar.dma_start(out=bqbo_sb[:, :], in_=bqbo[:, :])
            def wq_dma(lo, hi, eng):
                eng.dma_start(
                    out=wq_sb[:, lo:hi, :, :],
                    in_=wq[lo:hi, :, :, :].rearrange("d p a c -> p d a c"))

            wq_dma(0, 2, nc.sync)
            wq_dma(2, 4, nc.scalar)
            wq_dma(4, 6, nc.sync)
            wq_dma(6, 8, nc.scalar)
            nc.gpsimd.dma_start(
                out=wo_sb[:, :, :],
                in_=wo.rearrange("(a p) c -> p a c", p=128))

            # ---- persistent intermediates ----
            qT_sb = sb.tile([128, KC, SQ], f16, tag="qT")  # [dk(2 heads), dd, q]
            kT_sb = sb.tile([128, SK], f16, tag="kT")      # [dk(2 groups), w]
            vT_sb = sb.tile([128, SK], bf16, tag="vT")
            vt_t = [
                sb.tile([128, G, HD + 1], bf16, tag=f"vt{j}", name=f"vt{j}")
                for j in range(NJ)
            ]
            attnT = sb.tile([128, KC, SQ], bf16, tag="attnT")  # [dk(2 heads), pair, q]
            den = sb.tile([128, G, 3, SQ], f32, tag="den")
            den_rf = sb.tile([128, G, 3, SQ], f32, tag="denrf")
            den_r = sb.tile([128, G, 3, SQ], bf16, tag="denr")
            nc.vector.memset(den[:, :, :, :], 1.0)

            # ---- K/V projections, accumulated per contraction chunk ----
            tK = poolA.tile([128, 2, 512], f32, tag="pa", name="tK")
            tV = poolA.tile([128, 2, 512], f32, tag="pa", name="tV")
            for kc in range(KC):
                for h2 in range(2):
                    sl = slice(h2 * 384, (h2 + 1) * 384)
                    nc.tensor.matmul(tK[:, h2, 0:384], wk_sb[:, kc, :],
                                     xT_sb[:, kc, sl], start=(kc == 0), stop=False)
                    nc.tensor.matmul(tV[:, h2, 0:384], wv_sb[:, kc, :],
                                     xT_sb[:, kc, sl], start=(kc == 0), stop=False)
            for h2 in range(2):
                sl = slice(h2 * 384, (h2 + 1) * 384)
                nc.tensor.matmul(tK[:, h2, 0:384], wk_aug[:, :], xaug[:, sl],
                                 start=False, stop=True)
                nc.tensor.matmul(tV[:, h2, 0:384], wv_aug[:, :], xaug[:, sl],
                                 start=False, stop=True)
            nc.vector.tensor_copy(
                kT_sb.rearrange("p (a i) -> p a i", a=2), tK[:, :, 0:384])
            nc.vector.tensor_copy(
                vT_sb.rearrange("p (a i) -> p a i", a=2), tV[:, :, 0:384])

            # ---- V back to natural layout [w, dk], ones column appended ----
            for j in range(NJ):
                po = poolA.tile([128, 128], bf16, tag="pa", name=f"pstr{j}")
                nc.tensor.transpose(po, vT_sb[:, j * 128:(j + 1) * 128], ident)
                nc.vector.tensor_copy(
                    vt_t[j][:, :, 0:HD], po.rearrange("p (g d) -> p g d", g=G))
                nc.vector.tensor_copy(vt_t[j][:, :, HD:HD + 1], ones_sb[:, :])

            # ---- Q projection: all 8 dd blocks as one dense PE stream
            # (HAM warm-up ramp right before attention; keeps the pv ring
            # exclusively for attention so pairs alternate slots cleanly) ----
            def q_proj(dd, pool):
                ps = pool.tile([128, 512], f32, tag="pa" if pool is poolA else "pv",
                               name=f"psq{dd}")
                for kc in range(KC):
                    nc.tensor.matmul(
                        ps, wq_sb[:, dd, kc, :],
                        xT_sb[:, kc, HALF:HALF + SQ],
                        start=(kc == 0), stop=(kc == KC - 1),
                    )
                nc.scalar.activation(
                    qT_sb[:, dd, :], ps, mybir.ActivationFunctionType.Identity,
                    bias=bq_sb[:, dd:dd + 1],
                )

            for dd in range(KC):
                q_proj(dd, poolA)

            # ---- attention over head pairs, normalization in waves ----
            # Host permutes Wq columns so q dd-block p holds head p (group 0)
            # in rows 0:64 and head p+8 (group 1) in rows 64:128. A pair is
            # (head 8g+2pp, head 8g+2pp+1): same group, adjacent dd blocks.
            def norm_recip(c3):
                # approx recip (fp32) + bf16 cast: bf16 sel/den matmuls
                # stream 1 cyc/row vs ~1.5 for f32r HIGH mode. Waves 0/1
                # cast on the idle GpSimd; wave 2 is in the serial tail.
                eng = nc.vector if c3 == 2 else nc.gpsimd
                for gg2 in range(G):
                    nc.vector.reciprocal_approx_fast(
                        out=den_rf[:, gg2, c3, :], in_=den[:, gg2, c3, :])
                    # contiguous per-g rounding copy: each K=1 broadcast
                    # unblocks on its own half instead of a strided whole
                    eng.tensor_copy(den_r[:, gg2, c3, :], den_rf[:, gg2, c3, :])

            def norm_apply(plo, phi):
                # two K=1 broadcasts (selA->rows 0:64, selB->rows 64:128)
                # at the same legal base partition 32*(p%3)
                for p in range(plo, phi):
                    ps = poolA.tile([128, 512], f32, tag="pa", name=f"psn{p}")
                    k4 = 32 * (p % 3)
                    nc.tensor.matmul(ps, sel3_sb[k4:k4 + 1, 0, :],
                                     den_r[k4:k4 + 1, 0, p // 3, :],
                                     start=True, stop=False)
                    nc.tensor.matmul(ps, sel3_sb[k4:k4 + 1, 1, :],
                                     den_r[k4:k4 + 1, 1, p // 3, :],
                                     start=False, stop=True)
                    nc.vector.tensor_mul(attnT[:, p, :], attnT[:, p, :], ps)

            CSLOT = {0: 0, 2: 1, 1: 2}
            for pp_g in [(pp, g) for pp in range(4) for g in range(G)]:
                pp, g = pp_g
                pa = 2 * pp
                qrow = 64 * g
                kT_g = kT_sb[64 * g:64 * g + 64, :]
                pair = 4 * g + pp
                # pt per pair: [t, c, i(q), h2] bf16
                pt = ptp.tile([128, NT, 3, 2, 128], bf16, tag="pt", name=f"pt{pair}")
                pv = pvP.tile([128, NT, 2, 128], f32, tag="pv", name=f"pv{pair}")
                for hh in range(2):
                    t0 = 2 * hh
                    # per tile: 3 score matmuls -> exp -> band mask (c=0
                    # keeps kv_row >= q_col, c=1 all, c=2 <); PV of tile t
                    # then overlaps scores/exp of tile t+1.
                    # c-slot order (c=0, c=2, c=1): the two triangle-masked
                    # blocks sit contiguous so one DVE multiply covers them
                    # and the all-ones c=1 block needs no masking.
                    psc = {}
                    for t in (t0, t0 + 1):
                        psc[t] = poolA.tile([128, 4, 2, 128], f32, tag="pa",
                                            name=f"psc{pair}_{t}")
                        for c in range(3):
                            j = t + c
                            nc.tensor.matmul(
                                psc[t][:, CSLOT[c], :, :],
                                kT_g[:, 128 * j:128 * j + 128],
                                qT_sb[64 * g:64 * g + 64, pa:pa + 2,
                                      128 * t:128 * t + 128],
                                start=True, stop=True,
                            )
                        nc.scalar.activation(
                            pt[:, t], psc[t][:, 0:3, :, :],
                            mybir.ActivationFunctionType.Exp)
                        nc.vector.tensor_mul(pt[:, t, 0:2], pt[:, t, 0:2],
                                             mask_sb[:, 0, 0:2])
                    # PV: tiles t0/t0+1 share one PSUM bank; one start per bank
                    for t in (t0, t0 + 1):
                        for c in range(3):
                            j = t + c
                            nc.tensor.matmul(
                                pv[0:HD + 1, t, :, :],
                                vt_t[j][:, g, :],
                                pt[:, t, CSLOT[c], :, :],
                                start=(t == t0 and c == 0),
                                stop=(t == t0 + 1 and c == 2),
                            )
                if dbg:
                    nc.sync.dma_start(out=dbg_t["dbg_pt"][:, pair, :, :, :, :],
                                      in_=pt[:, :, :, :, :])
                nc.vector.tensor_copy(
                    attnT[qrow:qrow + 64, pa, :].rearrange(
                        "p (t i) -> p t i", t=NT),
                    pv[0:HD, :, 0, :])
                nc.scalar.activation(
                    attnT[qrow:qrow + 64, pa + 1, :].rearrange(
                        "p (t i) -> p t i", t=NT),
                    pv[0:HD, :, 1, :],
                    mybir.ActivationFunctionType.Identity)
                for h2 in range(2):
                    p_old = pa + h2
                    row = 32 * (p_old % 3)   # legal DVE base for both g
                    nc.vector.tensor_copy(
                        den[row:row + 1, g, p_old // 3, :].rearrange(
                            "p (t i) -> p t i", t=NT),
                        pv[HD:HD + 1, :, h2, :])
                if pp_g == (1, 1):
                    norm_recip(0)     # pairs p0..2 complete after (1, 1)
                if pp_g == (2, 0):
                    norm_apply(0, 1)
                if pp_g == (2, 1):
                    norm_apply(1, 3)
                    norm_recip(1)     # p3..5 complete after (2, 1)
                if pp_g == (3, 0):
                    norm_apply(3, 4)
                if pp_g == (3, 1):
                    norm_apply(4, 6)
            norm_recip(2)
            norm_apply(6, 8)

            if dbg:
                nc.sync.dma_start(out=dbg_t["dbg_qT"][:, :, :], in_=qT_sb[:, :, :])
                nc.sync.dma_start(out=dbg_t["dbg_kT"][:, :], in_=kT_sb[:, :])
                for j in range(NJ):
                    nc.sync.dma_start(out=dbg_t["dbg_vt"][:, j, :, :], in_=vt_t[j][:, :, :])
                nc.sync.dma_start(out=dbg_t["dbg_attnT"][:, :, :], in_=attnT[:, :, :])
                nc.sync.dma_start(out=dbg_t["dbg_den"][:, :, :], in_=den[:, :, :])

            # ---- output projection ----
            for do in range(KC):
                ps = poolA.tile([128, 512], f32, tag="pa", name=f"pso{do}")
                for p in range(KC):
                    nc.tensor.matmul(
                        ps, wo_sb[:, p, do * 128:(do + 1) * 128], attnT[:, p, :],
                        start=(p == 0), stop=(p == KC - 1),
                    )
                yt = yst.tile([128, SQ], bf16, tag="yt", name=f"yt{do}")
                nc.scalar.activation(yt, ps, mybir.ActivationFunctionType.Identity,
                                     bias=bo_sb[:, do:do + 1])
                # rotate store queues; keep scalar free for the biases
                eng = (nc.sync, nc.gpsimd)[do % 2]
                eng.dma_start(out=yT[do * 128:(do + 1) * 128, :], in_=yt[:, :])

    nc.finalize()
    return nc


def get_program():
    if "nc" not in _CACHE:
        _CACHE["nc"] = _build_program()
    return _CACHE["nc"]


def make_in_maps(x, Wq, bq, Wk, bk, Wv, bv, Wo, bo):
    """Host-side sharding: per-core input dicts."""
    import ml_dtypes

    bf = ml_dtypes.bfloat16
    x = np.ascontiguousarray(np.asarray(x, np.float32))
    wkb = np.concatenate([np.asarray(Wk, np.float32), np.asarray(bk, np.float32)[None]], 0)
    wvb = np.concatenate([np.asarray(Wv, np.float32), np.asarray(bv, np.float32)[None]], 0)
    sel3 = np.zeros((128, 2, 128), np.float32)
    sel3[0::32, 0, :64] = 1.0
    sel3[0::32, 1, 64:] = 1.0
    # head permutation: device column-block p holds [head p | head p+8]
    perm = np.empty(DIM, np.int64)
    for p in range(8):
        perm[128 * p:128 * p + 64] = np.arange(64 * p, 64 * p + 64)
        perm[128 * p + 64:128 * p + 128] = np.arange(64 * (p + 8), 64 * (p + 8) + 64)
    # triangle band mask per (tile-parity, chunk, q, head) block: c=0 keep
    # kv_row p >= q col i, c=1 all, c=2 keep p < i; same for every tile/head
    pi = np.arange(128)
    mask = np.ones((128, 2, 2, 2, 128), np.float32)
    mask[:, :, 0, :, :] = (pi[:, None] >= pi[None, :])[:, None, None, :]
    mask[:, :, 1, :, :] = (pi[:, None] < pi[None, :])[:, None, None, :]
    # device-layout packing: SBUF partition-major, contiguous per partition
    bq_p = np.asarray(bq, np.float32)[perm].reshape(KC, 128).T          # [p, dd]
    bo_p = np.asarray(bo, np.float32).reshape(KC, 128).T                # [p, do]
    cbf = np.concatenate(
        [np.ones((128, G), np.float32), np.eye(128, dtype=np.float32),
         mask.reshape(128, 1024), sel3.reshape(128, 256)], axis=1)
    common = {
        "wq": np.ascontiguousarray(
            np.asarray(Wq, np.float32)[:, perm].reshape(8, 128, 8, 128)
            .transpose(2, 1, 0, 3)).astype(np.float16),                 # [dd,p,kc,c]
        "wk": np.ascontiguousarray(
            wkb[:DIM].reshape(KC, 128, G * HD).transpose(1, 0, 2)).astype(np.float16),
        "wv": np.ascontiguousarray(
            wvb[:DIM].reshape(KC, 128, G * HD).transpose(1, 0, 2)).astype(np.float16),
        "wka": np.ascontiguousarray(wkb[DIM:DIM + 1]).astype(np.float16),
        "wva": np.ascontiguousarray(wvb[DIM:DIM + 1]).astype(np.float16),
        "wo": np.ascontiguousarray(np.asarray(Wo, np.float32)[perm, :]).astype(bf),
        "bqbo": np.ascontiguousarray(np.concatenate([bq_p, bo_p], axis=1)),
        "cbf": np.ascontiguousarray(cbf).astype(bf),
    }
    in_maps = []
    for c in range(NCORES):
        b, t = divmod(c, NCORES // BATCH)
        s0 = SQ * t
        xa = np.zeros((SK, DIM + 1), np.float32)
        lo, hi = max(0, s0 - HALF), min(SEQ, s0 + SQ + HALF)
        xa[lo - (s0 - HALF):hi - (s0 - HALF), :DIM] = x[b, lo:hi]
        xa[lo - (s0 - HALF):hi - (s0 - HALF), DIM] = 1.0
        in_maps.append({"xaT": np.ascontiguousarray(xa.T).astype(np.float16), **common})
    return in_maps


def assemble_output(results):
    y = np.empty((BATCH, SEQ, DIM), np.float32)
    for c in range(NCORES):
        b, t = divmod(c, NCORES // BATCH)
        y[b, SQ * t:SQ * (t + 1), :] = results[c]["yT"].T.astype(np.float32)
    return y


def kernel(**inputs):
    from concourse.bass_utils import run_bass_kernel_spmd

    nc = get_program()
    in_maps = make_in_maps(**inputs)
    last_err = None
    for _ in range(3):  # retry: transient NRT device wedges recover on rerun
        try:
            res = run_bass_kernel_spmd(nc, in_maps, list(range(NCORES)))
            return assemble_output(res.results)
        except Exception as e:  # noqa: BLE001
            last_err = e
    raise last_err

